# revision 21
# baseline (speedup 1.0000x reference)
"""AreaAttention Trainium2 kernel: 8-core data-parallel over (batch, area) slabs.

B=4, H=W=64, C=256, AREA=4 -> 16 independent slabs of 16 image rows (1024
positions each). Core c owns slabs 2c, 2c+1. Only cross-slab coupling is the
5x5 depthwise-conv halo (2 rows), shipped by the host inside each slab's padded
input. No collectives.

v3 pipeline (ACT is the pacer: 128 exp instructions ~= 145us):
  x arrives via straight DMA and is transposed on the PE. The attention runs
  as a 4-group software pipeline -- scores+exp of group n+1 interleave with
  attnv+key-sums of group n so the ACT stream never gaps; qk / v / proj
  matmuls are woven into the PSUM O/U windows between groups. The epilogue
  is split: recip+mul (releases the PSUM accumulators, emitted early on DVE)
  and the +pe add (GPSIMD, emitted late) so the DVE queue never head-blocks
  a PSUM release behind the conv chains. The depthwise conv runs off the PE:
  21 taps as TS(4x)+TT(2x) pairs on DVE reading an even-aligned buffer
  (odd-dx taps read a 1-shifted copy), 4 taps on GPSIMD via broadcast-AP
  tensor_tensor pairs.
"""

import sys

import numpy as np
import ml_dtypes

if "/opt/trn_rl_repo" not in sys.path:
    sys.path.insert(0, "/opt/trn_rl_repo")

BF16 = ml_dtypes.bfloat16

B, H, W, C = 4, 64, 64, 256
AREA, NH, HD = 4, 8, 32
EPS = 1e-3
N_CORES = 8
SLABS = 2          # slabs per core
ROWS = 16          # image rows per slab
HROWS = ROWS + 4   # with conv halo
POS = ROWS * W     # 1024 valid positions
HPOS = HROWS * W   # 1280 padded positions
PW = W + 4         # padded conv row width (68)
PADG = 4           # guard cols at both ends of the padded conv buffer
VTC = PADG + HROWS * PW + PADG   # 1368 cols of conv input buffer
ACCW = ROWS * PW   # 1088 conv output cols (includes margin cols)
SCALE = 1.0 / float(np.sqrt(np.float32(HD)))

ALL_TAPS = [(dy, dx) for dy in range(-2, 3) for dx in range(-2, 3)]
EVEN_TAPS = [t for t in ALL_TAPS if t[1] % 2 == 0]   # 15
ODD_TAPS = [t for t in ALL_TAPS if t[1] % 2 != 0]    # 10
# GPSIMD takes some odd-dx taps (slab-1 groups get more: their deadline is
# later); DVE takes the rest, reading a 1-shifted copy for its odd-dx taps
# so every DVE access is even-aligned (4x/2x DVE modes).
def dve_taps(s):
    return EVEN_TAPS + ODD_TAPS

_NC_CACHE = {}


def _affine(g, b, m, v):
    g, b, m, v = (np.asarray(a, np.float32) for a in (g, b, m, v))
    s = (g / np.sqrt(v + EPS)).astype(np.float32)
    t = (b - m * s).astype(np.float32)
    return s, t


def _build_nc():
    import concourse.bass as bass
    import concourse.tile as tile
    import concourse.mybir as mybir
    from concourse import bacc
    from contextlib import ExitStack

    f32 = mybir.dt.float32
    bf16 = mybir.dt.bfloat16
    Alu = mybir.AluOpType
    Act = mybir.ActivationFunctionType

    nc = bacc.Bacc("TRN2", target_bir_lowering=False, debug=False,
                   num_devices=N_CORES)

    x_d = nc.declare_dram_parameter("x", [SLABS * HPOS, C], bf16, isOutput=False)
    wqk_d = nc.declare_dram_parameter("wqk", [C, 2 * C], bf16, isOutput=False)
    id_d = nc.declare_dram_parameter("ident", [128, 128], bf16, isOutput=False)
    wv_d = nc.declare_dram_parameter("wv", [C, C], bf16, isOutput=False)
    wproj_d = nc.declare_dram_parameter("wproj", [C, C], bf16, isOutput=False)
    vecs_d = nc.declare_dram_parameter("vecs", [128, 22], f32, isOutput=False)
    wpe_d = nc.declare_dram_parameter("wpe", [128, 50], f32, isOutput=False)
    tp_d = nc.declare_dram_parameter("tp", [C], f32, isOutput=False)
    out_d = nc.declare_dram_parameter("out", [SLABS * POS, C], f32, isOutput=True)

    # vecs columns: sqk[0:4] tqk[4:8] tv[8:10] tvpe[10:12] (scratch)[12:14]
    # tvedge[14:22]
    COL_SQK, COL_TQK, COL_TV, COL_TVPE, COL_EDGE = 0, 4, 8, 10, 14

    with tile.TileContext(nc) as tc, ExitStack() as est:
        consts = est.enter_context(tc.tile_pool(name="consts", bufs=1))
        xs_p = est.enter_context(tc.tile_pool(name="xs", bufs=2))
        xt_p = est.enter_context(tc.tile_pool(name="xt", bufs=4))
        qkt_p = est.enter_context(tc.tile_pool(name="qkt", bufs=8))
        vsb_p = est.enter_context(tc.tile_pool(name="vsb", bufs=8))
        vt_p = est.enter_context(tc.tile_pool(name="vt", bufs=4))
        vto_p = est.enter_context(tc.tile_pool(name="vto", bufs=4))
        accd_p = est.enter_context(tc.tile_pool(name="accd", bufs=2))
        dtmp_p = est.enter_context(tc.tile_pool(name="dtmp", bufs=2))
        pes_p = est.enter_context(tc.tile_pool(name="pes", bufs=4))
        exps_p = est.enter_context(tc.tile_pool(name="exps", bufs=24))
        inv_p = est.enter_context(tc.tile_pool(name="inv", bufs=2))
        on_p = est.enter_context(tc.tile_pool(name="on", bufs=2))
        prhs_p = est.enter_context(tc.tile_pool(name="prhs", bufs=4))
        out_p = est.enter_context(tc.tile_pool(name="outp", bufs=4))

        # Unified PSUM pool: tags S (2 slots = 4 banks), O (2 banks),
        # U (2 banks). The exp stream owns S; attnv/sums own O/U while a
        # group is accumulating; prep/proj rotate through O/U in the gaps.
        ps = est.enter_context(tc.tile_pool(name="ps", bufs=2, space="PSUM"))

        # ---- constants (ident + x + wqk first: they gate the PE pipeline)
        ident = consts.tile([128, 128], bf16, tag="ident", name="ident")
        nc.sync.dma_start(out=ident[:], in_=id_d[:, :])

        # x slabs: straight (fast) DMA into [128, 10*256] staging tiles;
        # slab 0 is split so its first transposes start sooner
        x_ap = x_d.ap() if hasattr(x_d, "ap") else x_d[:]

        def stage_dma(xst, s, t0, nt):
            src = bass.AP(
                tensor=x_ap.tensor,
                offset=x_ap.offset + (s * HPOS + t0 * 128) * C,
                ap=[[C, 128], [128 * C, nt], [1, C]],
            )
            nc.sync.dma_start(
                out=xst[:, t0 * C:(t0 + nt) * C].rearrange(
                    "p (t c) -> p t c", c=C),
                in_=src)

        xstage = []
        for s in range(SLABS):
            xst = xs_p.tile([128, 10 * C], bf16, tag="xs", name=f"xs{s}")
            xstage.append(xst)
        stage_dma(xstage[0], 0, 0, 5)
        wqk_sb = []
        for k in range(2):
            wq = consts.tile([128, 2 * C], bf16, tag=f"wqk{k}", name=f"wqk{k}")
            nc.sync.dma_start(out=wq[:], in_=wqk_d[k * 128:(k + 1) * 128, :])
            wqk_sb.append(wq)
        stage_dma(xstage[0], 0, 5, 5)
        stage_dma(xstage[1], 1, 0, 5)
        stage_dma(xstage[1], 1, 5, 5)

        vecs = consts.tile([128, 22], f32, tag="vecs", name="vecs")
        wpe = consts.tile([128, 50], f32, tag="wpe", name="wpe")
        tpb = consts.tile([128, C], f32, tag="tpb", name="tpb")
        wv_sb, wproj_sb = [], []
        for k in range(2):
            wv_t = consts.tile([128, C], bf16, tag=f"wv{k}", name=f"wv{k}")
            nc.sync.dma_start(out=wv_t[:], in_=wv_d[k * 128:(k + 1) * 128, :])
            wv_sb.append(wv_t)
            wp = consts.tile([128, C], bf16, tag=f"wproj{k}", name=f"wproj{k}")
            nc.sync.dma_start(out=wp[:], in_=wproj_d[k * 128:(k + 1) * 128, :])
            wproj_sb.append(wp)
        nc.sync.dma_start(out=vecs[:], in_=vecs_d[:, :])
        nc.sync.dma_start(out=wpe[:], in_=wpe_d[:, :])
        tp_ap = tp_d.ap() if hasattr(tp_d, "ap") else tp_d[:]
        nc.sync.dma_start(
            out=tpb[:],
            in_=bass.AP(tensor=tp_ap.tensor, offset=tp_ap.offset,
                        ap=[[0, 128], [1, C]]),
        )

        ones = consts.tile([128, HD], bf16, tag="ones", name="ones")
        nc.vector.memset(ones[:], 1.0)
        # dummy exp to hoist the ACT exp-table load off the critical path;
        # writes an unused-but-read vecs column so the BIR verifier is happy
        nc.scalar.activation(vecs[:, 12:13], ones[:, 0:1], Act.Exp)

        xt = [[None] * 2 for _ in range(SLABS)]
        qkt = [[None] * 4 for _ in range(SLABS)]
        vsb = [[None] * 4 for _ in range(SLABS)]   # j-pairs: [128, 512]
        vt = [[None] * 2 for _ in range(SLABS)]
        vto = [[None] * 2 for _ in range(SLABS)]
        pess = [[None] * 2 for _ in range(SLABS)]
        prhs = [[None] * 2 for _ in range(SLABS)]

        def emit_xt(s, ks=(0, 1)):
            """PE-transpose the staged x slab into xt[s][k] = [128 c, HPOS].

            bf16 PSUM tiles (1 bank for 1024 cols). Slab-0 copies ride the
            still-idle ACT engine; slab-1 copies go to DVE.
            """
            for k in ks:
                xtt = xt_p.tile([128, HPOS], bf16, tag="xt", name=f"xt{s}{k}")
                for t0, nt, tg in ((0, 8, "O"), (8, 2, "U")):
                    pst = ps.tile([128, nt * 128], bf16, tag=tg, bufs=1,
                                  name="pst")
                    for t in range(nt):
                        nc.tensor.matmul(
                            pst[:, t * 128:(t + 1) * 128],
                            xstage[s][:, (t0 + t) * C + k * 128:
                                      (t0 + t) * C + k * 128 + 128],
                            ident[:],
                            is_transpose=True, start=True, stop=True,
                        )
                    if s == 0:
                        nc.scalar.copy(
                            xtt[:, t0 * 128:(t0 + nt) * 128], pst[:])
                    else:
                        nc.vector.tensor_copy(
                            xtt[:, t0 * 128:(t0 + nt) * 128], pst[:])
                xt[s][k] = xtt

        def emit_qk(s, ms, tags):
            for m, tg in zip(ms, tags):
                qt = qkt_p.tile([128, POS], bf16, tag="qkt", name=f"qkt{s}{m}")
                psq = ps.tile([128, POS], f32, tag=tg,
                              bufs=(2 if tg == "S" else 1), name="psq")
                for pt in range(2):
                    for k in range(2):
                        nc.tensor.matmul(
                            psq[:, pt * 512:(pt + 1) * 512],
                            wqk_sb[k][:, m * 128:(m + 1) * 128],
                            xt[s][k][:, 2 * W + pt * 512: 2 * W + (pt + 1) * 512],
                            start=(k == 0), stop=(k == 1),
                        )
                nc.vector.tensor_scalar(
                    out=qt[:], in0=psq[:],
                    scalar1=vecs[:, COL_SQK + m:COL_SQK + m + 1],
                    scalar2=vecs[:, COL_TQK + m:COL_TQK + m + 1],
                    op0=Alu.mult, op1=Alu.add,
                )
                qkt[s][m] = qt

        def emit_psv(s, jp, tag):
            # v [pos, c], two j-chunks per PSUM tile to batch the casts
            psv = ps.tile([128, 1024], f32, tag=tag, bufs=1, name="psv")
            for half in range(2):
                j = jp * 2 + half
                for k in range(2):
                    nc.tensor.matmul(
                        psv[:, half * 512: half * 512 + C],
                        xt[s][k][:, 2 * W + j * 128: 2 * W + (j + 1) * 128],
                        wv_sb[k][:],
                        start=(k == 0), stop=(k == 1),
                    )
            vj = vsb_p.tile([128, 512], bf16, tag="vsb", name=f"v{s}{jp}")
            nc.scalar.copy(
                vj[:].rearrange("p (h c) -> p h c", c=C),
                psv[:].rearrange("p (h c) -> p h c", c=512)[:, :, 0:C])
            vsb[s][jp] = vj

        def emit_vt_alloc(s):
            for m in range(2):
                vtt = vt_p.tile([128, VTC], bf16, tag="vt", name=f"vt{s}{m}")
                nc.gpsimd.memset(vtt[:], 0.0)
                vt[s][m] = vtt

        def emit_vt_pt(s, m, pt, tag):
            n = 512 if pt < 2 else 256
            psvt = ps.tile([128, POS], f32, tag=tag, bufs=1, name="psvt")
            for k in range(2):
                nc.tensor.matmul(
                    psvt[:, :n],
                    wv_sb[k][:, m * 128:(m + 1) * 128],
                    xt[s][k][:, pt * 512: pt * 512 + n],
                    start=(k == 0), stop=(k == 1),
                )
            vtt = vt[s][m]
            r0 = pt * 8
            nrows = n // W
            cur = r0
            while cur < r0 + nrows:
                if cur < 2:
                    end = min(2, r0 + nrows)
                    bias_col = COL_EDGE + s * 4 + 0 * 2 + m
                elif cur < HROWS - 2:
                    end = min(HROWS - 2, r0 + nrows)
                    bias_col = COL_TV + m
                else:
                    end = r0 + nrows
                    bias_col = COL_EDGE + s * 4 + 1 * 2 + m
                nr = end - cur
                srcv = psvt[:, :n].rearrange("p (r w) -> p r w", w=W)[
                    :, cur - r0:end - r0, :]
                dstv = bass.AP(
                    tensor=vtt.tensor,
                    offset=vtt.offset + PADG + cur * PW + 2,
                    ap=[vtt.ap[0], [PW, nr], [1, W]],
                )
                nc.vector.tensor_scalar(
                    out=dstv, in0=srcv,
                    scalar1=vecs[:, bias_col:bias_col + 1],
                    scalar2=None, op0=Alu.add,
                )
                cur = end

        def wcol_ap(dy, dx, q):
            wcol = ((dy + 2) * 5 + (dx + 2)) * 2 + q
            return wpe[:, wcol:wcol + 1]

        ACCD = {}

        def emit_conv_c1(s, q):
            """even-dx taps (chunk 1) as TS(4x)+TT(2x) pairs into acc.

            s_pe is folded into wpe on the host; the first tap bakes in the
            tvpe bias via tensor_scalar's second op.
            """
            vtt = vt[s][q]
            acc = accd_p.tile([128, ACCW], bf16, tag="accd", name=f"ad{s}{q}")
            for t, (dy, dx) in enumerate(EVEN_TAPS):
                off = PADG + (2 + dy) * PW + dx
                src = vtt[:, off:off + ACCW]
                w = wcol_ap(dy, dx, q)
                if t == 0:
                    nc.vector.tensor_scalar(
                        out=acc[:], in0=src, scalar1=w,
                        scalar2=vecs[:, COL_TVPE + q:COL_TVPE + q + 1],
                        op0=Alu.mult, op1=Alu.add)
                else:
                    tmp = dtmp_p.tile([128, ACCW], bf16, tag="dtmp",
                                      name="dtmp")
                    nc.vector.tensor_scalar_mul(tmp[:], src, w)
                    nc.vector.tensor_add(acc[:], acc[:], tmp[:])
            ACCD[(s, q)] = acc

        def emit_conv_c2(s, q, fence=None):
            """odd-dx taps (chunk 2): reads the 1-shifted copy vto so every
            DVE access is even-aligned; last add writes pes. `fence` orders
            the chunk after a latency-critical DVE op (the scheduler
            otherwise reorders by readiness under its own cost model)."""
            from concourse.tile import add_dep_helper
            vtt = vt[s][q]
            vo = vto_p.tile([128, VTC], bf16, tag="vto", name=f"vto{s}{q}")
            cp = nc.vector.tensor_copy(vo[:, 0:VTC - 1], vtt[:, 1:VTC])
            if fence is not None:
                add_dep_helper(cp.ins, fence.ins, sync=False,
                               reason="dve deadline order")
            vto[s][q] = vo
            acc = ACCD[(s, q)]
            pes = pes_p.tile([128, ACCW], bf16, tag="pes", name=f"pes{s}{q}")
            n = len(ODD_TAPS)
            for t, (dy, dx) in enumerate(ODD_TAPS):
                off = PADG + (2 + dy) * PW + dx
                src = vo[:, off - 1:off - 1 + ACCW]
                w = wcol_ap(dy, dx, q)
                tmp = dtmp_p.tile([128, ACCW], bf16, tag="dtmp", name="dtmp")
                nc.vector.tensor_scalar_mul(tmp[:], src, w)
                nc.vector.tensor_add(
                    (pes[:] if t == n - 1 else acc[:]), acc[:], tmp[:])
            pess[s][q] = pes

        # ---- attention pipeline pieces ----
        OSU = {}
        EX = {}

        def emit_A(s, q, j):
            """scores + exp for key-chunk j of group (s, q)."""
            kT = qkt[s][2 + q]
            qT = qkt[s][q]
            for hp in range(2):
                for hf in range(2):
                    Sp = ps.tile([128, 1024], f32, tag="S", name="Sp")
                    for e in range(2):
                        h = 2 * hp + e
                        nc.tensor.matmul(
                            Sp[:, e * 512:(e + 1) * 512],
                            kT[32 * h:32 * (h + 1), j * 128:(j + 1) * 128],
                            qT[32 * h:32 * (h + 1), hf * 512:(hf + 1) * 512],
                            start=True, stop=True,
                            tile_position=(32 * h, 0),
                        )
                    ext = exps_p.tile([128, 1024], bf16, tag="ex", name="ext")
                    nc.scalar.activation(ext[:], Sp[:], Act.Exp, scale=SCALE)
                    EX[(s, q, j, hf, hp)] = ext

        def emit_B(s, q, j, sums_first=False):
            """attnv + key-sums for key-chunk j of group (s, q)."""
            if j == 0:
                O = ps.tile([128, POS], f32, tag="O", bufs=1, name="O")
                SU = ps.tile([128, POS], f32, tag="U", bufs=1, name="U")
                OSU[(s, q)] = (O, SU)
            O, SU = OSU[(s, q)]
            vj = vsb[s][j // 2]

            def emit_attnv():
                for h in range(4):
                    for hf in range(2):
                        nc.tensor.matmul(
                            O[32 * h:32 * (h + 1), hf * 512:(hf + 1) * 512],
                            vj[:, (j % 2) * C + q * 128 + 32 * h:
                               (j % 2) * C + q * 128 + 32 * (h + 1)],
                            EX[(s, q, j, hf, h // 2)][:,
                                                      (h % 2) * 512:
                                                      (h % 2 + 1) * 512],
                            start=(j == 0), stop=(j == 7),
                            tile_position=(0, 32 * h),
                            skip_group_check=True,
                        )

            def emit_sums():
                for h in range(4):
                    for hf in range(2):
                        nc.tensor.matmul(
                            SU[32 * h:32 * (h + 1), hf * 512:(hf + 1) * 512],
                            ones[:],
                            EX[(s, q, j, hf, h // 2)][:,
                                                      (h % 2) * 512:
                                                      (h % 2 + 1) * 512],
                            start=(j == 0), stop=(j == 7),
                            tile_position=(0, 32 * h),
                            skip_group_check=True,
                        )

            order = ((emit_sums, emit_attnv) if sums_first
                     else (emit_attnv, emit_sums))
            for fn in order:
                fn()

        ONQ = {}

        def emit_epi_rel(s, q):
            """normalize: releases the O/U PSUM banks; keep this early on
            the DVE queue."""
            O, SU = OSU[(s, q)]
            invq = inv_p.tile([128, POS], f32, tag="inv", name="invq")
            nc.vector.reciprocal_approx_fast(invq[:], SU[:])
            onq = on_p.tile([128, POS], f32, tag="on", name="onq")
            mul_ins = nc.vector.tensor_mul(onq[:], O[:], invq[:])
            ONQ[(s, q)] = onq
            return mul_ins

        def emit_epi_add(s, q, on_gp=False):
            """pr = attn_out + pe (SBUF-only operands)."""
            onq = ONQ[(s, q)]
            pes = pess[s][q]
            pr = prhs_p.tile([128, POS], bf16, tag="prhs", name=f"prhs{s}{q}")
            pes_ap = bass.AP(
                tensor=pes.tensor,
                offset=pes.offset + 2,
                ap=[pes.ap[0], [PW, ROWS], [1, W]],
            )
            eng = nc.gpsimd if on_gp else nc.vector
            eng.tensor_add(
                pr[:].rearrange("p (r w) -> p r w", w=W),
                onq[:].rearrange("p (r w) -> p r w", w=W),
                pes_ap,
            )
            prhs[s][q] = pr

        def emit_proj_p(s, p, tag):
            pse = ps.tile([128, C], f32, tag=tag,
                          bufs=(2 if tag == "S" else 1), name="pse")
            for k in range(2):
                nc.tensor.matmul(
                    pse[:],
                    prhs[s][k][:, p * 128:(p + 1) * 128],
                    wproj_sb[k][:],
                    start=(k == 0), stop=(k == 1),
                )
            ot = out_p.tile([128, C], f32, tag="ot", name="ot")
            nc.vector.tensor_add(ot[:], pse[:], tpb[:])
            nc.sync.dma_start(
                out=out_d[s * POS + p * 128: s * POS + (p + 1) * 128, :],
                in_=ot[:],
            )

        def emit_proj(s, tags):
            for p in range(8):
                emit_proj_p(s, p, tags[p % len(tags)])

        # ================= emission program =================
        # The Tile scheduler list-schedules per engine by (readiness,
        # emission priority) under its own cost model -- emission order is
        # a preference, not a guarantee. Deadline-critical DVE ordering is
        # enforced with explicit fences (conv chains after the epi_rel that
        # releases the PSUM accumulators).

        emit_xt(0)
        emit_qk(0, ms=(0, 2), tags=("S", "S"))
        emit_A(0, 0, 0)
        emit_xt(1, ks=(0,))
        emit_A(0, 0, 1)
        emit_xt(1, ks=(1,))
        emit_qk(0, ms=(1, 3), tags=("S", "S"))
        emit_vt_alloc(0)
        emit_A(0, 0, 2)
        emit_psv(0, 0, "U")
        emit_psv(0, 1, "O")
        emit_A(0, 0, 3)
        emit_vt_pt(0, 0, 0, "U")
        emit_vt_pt(0, 0, 1, "O")
        emit_A(0, 0, 4)
        emit_vt_pt(0, 0, 2, "U")
        emit_vt_pt(0, 1, 0, "O")
        emit_A(0, 0, 5)
        emit_vt_pt(0, 1, 1, "U")
        emit_vt_pt(0, 1, 2, "O")
        emit_A(0, 0, 6)
        emit_qk(1, ms=(0, 2), tags=("U", "O"))
        emit_A(0, 0, 7)
        emit_qk(1, ms=(1, 3), tags=("U", "O"))
        emit_psv(0, 2, "U")
        emit_psv(0, 3, "O")
        emit_psv(1, 0, "U")
        emit_psv(1, 1, "O")
        emit_psv(1, 2, "U")
        emit_psv(1, 3, "O")

        emit_vt_alloc(1)   # gp memsets queued early
        emit_B(0, 0, 0)
        emit_B(0, 0, 1)
        emit_B(0, 0, 2)
        emit_B(0, 0, 3)
        emit_conv_c1(0, 0)

        # --- weave A(0,1) with tail of B(0,0); epi frees O/U; vt(1) prep ---
        emit_A(0, 1, 0)
        emit_B(0, 0, 4)
        emit_A(0, 1, 1)
        emit_B(0, 0, 5)
        emit_A(0, 1, 2)
        emit_B(0, 0, 6)
        emit_A(0, 1, 3)
        emit_B(0, 0, 7, sums_first=True)
        f00 = emit_epi_rel(0, 0)
        emit_A(0, 1, 4)
        emit_vt_pt(1, 0, 0, "O")
        emit_vt_pt(1, 0, 1, "U")
        emit_A(0, 1, 5)
        emit_vt_pt(1, 0, 2, "O")
        emit_vt_pt(1, 1, 0, "U")
        emit_A(0, 1, 6)
        emit_vt_pt(1, 1, 1, "O")
        emit_vt_pt(1, 1, 2, "U")
        emit_A(0, 1, 7)
        emit_conv_c2(0, 0, fence=f00)
        emit_B(0, 1, 0)
        emit_B(0, 1, 1)
        emit_B(0, 1, 2)
        emit_B(0, 1, 3)
        emit_epi_add(0, 0, on_gp=True)
        emit_conv_c1(0, 1)

        # --- weave A(1,0) with tail of B(0,1) ---
        emit_A(1, 0, 0)
        emit_B(0, 1, 4)
        emit_A(1, 0, 1)
        emit_B(0, 1, 5)
        emit_A(1, 0, 2)
        emit_B(0, 1, 6)
        emit_A(1, 0, 3)
        emit_B(0, 1, 7, sums_first=True)
        f01 = emit_epi_rel(0, 1)
        emit_conv_c2(0, 1, fence=f01)
        emit_epi_add(0, 1, on_gp=True)
        emit_A(1, 0, 4)
        emit_B(1, 0, 0)
        emit_A(1, 0, 5)
        emit_B(1, 0, 1)
        emit_A(1, 0, 6)
        emit_B(1, 0, 2)
        emit_A(1, 0, 7)
        emit_B(1, 0, 3)
        emit_conv_c1(1, 0)
        emit_conv_c1(1, 1)

        # --- weave A(1,1) with tail of B(1,0); proj(0) in the G3 window ---
        emit_A(1, 1, 0)
        emit_B(1, 0, 4)
        emit_A(1, 1, 1)
        emit_B(1, 0, 5)
        emit_A(1, 1, 2)
        emit_B(1, 0, 6)
        emit_A(1, 1, 3)
        emit_B(1, 0, 7, sums_first=True)
        f10 = emit_epi_rel(1, 0)
        emit_conv_c2(1, 0, fence=f10)
        emit_epi_add(1, 0, on_gp=True)
        emit_A(1, 1, 4)
        emit_proj_p(0, 0, "O")
        emit_proj_p(0, 1, "U")
        emit_A(1, 1, 5)
        emit_proj_p(0, 2, "O")
        emit_proj_p(0, 3, "U")
        emit_A(1, 1, 6)
        emit_proj_p(0, 4, "O")
        emit_proj_p(0, 5, "U")
        emit_A(1, 1, 7)
        emit_proj_p(0, 6, "O")
        emit_proj_p(0, 7, "U")
        emit_B(1, 1, 0)
        emit_B(1, 1, 1)
        emit_B(1, 1, 2)
        emit_B(1, 1, 3)
        emit_conv_c2(1, 1)
        emit_B(1, 1, 4)
        emit_B(1, 1, 5)
        emit_B(1, 1, 6)
        emit_B(1, 1, 7, sums_first=True)
        emit_epi_rel(1, 1)
        emit_epi_add(1, 1)
        emit_proj(1, tags=("S", "S", "O", "U"))

    nc.compile()
    return nc


def prep_inputs(inputs):
    """Host-side weight folding + per-core shard construction."""
    s_qk, t_qk = _affine(inputs["g_qk"], inputs["b_qk"], inputs["m_qk"], inputs["v_qk"])
    s_v, t_v = _affine(inputs["g_v"], inputs["b_v"], inputs["m_v"], inputs["v_v"])
    s_pe, t_pe = _affine(inputs["g_pe"], inputs["b_pe"], inputs["m_pe"], inputs["v_pe"])
    s_p, t_p = _affine(inputs["g_proj"], inputs["b_proj"], inputs["m_proj"],
                       inputs["v_proj"])

    wqk = np.ascontiguousarray(np.asarray(inputs["w_qk"], np.float32)).astype(BF16)
    wv = np.ascontiguousarray(
        np.asarray(inputs["w_v"], np.float32) * s_v[None, :]).astype(BF16)
    wproj = np.ascontiguousarray(
        np.asarray(inputs["w_proj"], np.float32) * s_p[None, :]).astype(BF16)

    # conv weights with the pe BN scale folded in: pes = conv_wpe'(vt) + tvpe
    wpe_r = np.asarray(inputs["w_pe"], np.float32).reshape(25, C)
    wpe_t = np.zeros((128, 50), np.float32)
    for tap in range(25):
        for m in range(2):
            wpe_t[:, tap * 2 + m] = (wpe_r[tap, m * 128:(m + 1) * 128]
                                     * s_pe[m * 128:(m + 1) * 128])

    ident = np.eye(128, dtype=np.float32).astype(BF16)


    tvpe = t_v + t_pe
    x = np.asarray(inputs["x"], np.float32).reshape(B, H, W, C)

    in_maps = []
    for c in range(N_CORES):
        vecs = np.zeros((128, 22), np.float32)
        for m in range(4):
            vecs[:, 0 + m] = s_qk[m * 128:(m + 1) * 128]
            vecs[:, 4 + m] = t_qk[m * 128:(m + 1) * 128]
        for m in range(2):
            vecs[:, 8 + m] = t_v[m * 128:(m + 1) * 128]
            vecs[:, 10 + m] = tvpe[m * 128:(m + 1) * 128]

        xs = np.zeros((SLABS, HROWS, W, C), np.float32)
        for k in range(SLABS):
            slab = 2 * c + k
            b, a = divmod(slab, AREA)
            r0 = a * ROWS - 2
            for r in range(HROWS):
                rr = r0 + r
                if 0 <= rr < H:
                    xs[k, r] = x[b, rr]
            for e, oob in ((0, a == 0), (1, a == AREA - 1)):
                if not oob:
                    for m in range(2):
                        vecs[:, 14 + k * 4 + e * 2 + m] = t_v[m * 128:(m + 1) * 128]

        in_maps.append({
            "x": np.ascontiguousarray(xs.reshape(SLABS * HPOS, C)).astype(BF16),
            "wqk": wqk, "ident": ident, "wv": wv, "wproj": wproj,
            "vecs": vecs, "wpe": wpe_t,
            "tp": t_p.astype(np.float32),
        })
    return in_maps


def assemble_output(results):
    out = np.empty((B, AREA * POS, C), np.float32)
    for c in range(N_CORES):
        r = np.asarray(results[c]["out"]).reshape(SLABS, POS, C)
        for k in range(SLABS):
            slab = 2 * c + k
            b, a = divmod(slab, AREA)
            out[b, a * POS:(a + 1) * POS] = r[k]
    return out.reshape(B, H, W, C)


def get_nc():
    if "nc" not in _NC_CACHE:
        _NC_CACHE["nc"] = _build_nc()
    return _NC_CACHE["nc"]


def kernel(**inputs):
    import time as _time

    from concourse.bass_utils import run_bass_kernel_spmd

    nc = get_nc()
    in_maps = prep_inputs(inputs)
    last_err = None
    for attempt in range(3):
        try:
            res = run_bass_kernel_spmd(nc, in_maps, core_ids=list(range(N_CORES)))
            return assemble_output(res.results)
        except Exception as e:  # rare transient device faults; device recovers
            last_err = e
            _time.sleep(2.0)
    raise last_err


if __name__ == "__main__":
    get_nc()
    print("built + compiled OK")
